# revision 4
# baseline (speedup 1.0000x reference)
"""Trainium2 Bass kernel for nn_DiscreteContinuousConv2d (sparse DISCO conv).

Math (see reference):
    xq   = x * quadrature_weights               (B, C, n_in)
    xk   = segment_sum(xq[psi_idx_in] * psi_vals, k*n_out + psi_idx_out)
    out  = einsum("knbc,ock->bon", xk, weight) + bias

Kernel reformulation (Y-form): fold the channel/kernel mixing BEFORE the
sparse contraction.  With
    U[i, k, b, oc] = sum_c x[b, c, i] * weight[oc, c, k]        (dense, on PE)
and val'[e] = psi_vals[e] * qw[psi_idx_in[e]], the output is a pure
gather/segment-sum over the sparse entries:
    out[b, oc, o] = sum_{e in bin o} val'[e] * U[idx_in[e], idx_k[e], b, oc] + bias

Distribution: output-sharded over the 8 cores (core r owns o in
[r*2048, (r+1)*2048)); entries are bucketed per core on the host; no
collectives.  Per core:
    1. PE builds the U table (fp16, 16384*9 rows x 64) -> DRAM.
    2. Per 128-bin o-tile: dma_gather fetches each entry's U row.  The DMA
       gather element is 256B = a PAIR of adjacent U rows (j'//2); entries are
       parity-sorted per (o-tile, group) so a slot's used half (A = even row,
       B = odd row) is a STATIC slice of the gathered pair.  Because gather
       indices are int16, entries are grouped by idx_in>>12 (4 groups) and
       the gather reads from a per-group base offset.  The 4 group gathers
       go to 4 SWDGE queues, each served by its own Q7 core pair, so the
       descriptor generation (the dominant cost) runs 4-wide.
    3. The segment-sum one-hot, with val' (and the fp16 2^14 scale) folded
       in, is built on the HOST and DMA'd: PE matmuls (lhsT=one-hot val,
       rhs=gathered half) accumulate the 128-bin segment sums in PSUM;
       +bias, *2^-14 unscale; DMA out.

Host-side work is limited to index/layout preprocessing of the sparse
pattern (bucket/sort/pad, fold quadrature weights into per-entry one-hot
values) and the final unshard.
"""

import numpy as np
from contextlib import ExitStack

import concourse.bass as bass
import concourse.mybir as mybir
import concourse.tile as tile
from concourse import bacc
from concourse.bass_utils import run_bass_kernel_spmd

P = 128
N_CORES = 8
B, C, OC, K = 2, 32, 32, 9
N_IN = 16384
N_OUT = 16384
O_PER_CORE = N_OUT // N_CORES          # 2048
O_TILES = O_PER_CORE // P              # 16
V64 = B * OC                           # 64 values per U row
KV = K * V64                           # 576
N_GRP = 4                              # idx_in >> 12 gather groups
I_GRP = N_IN // N_GRP                  # 4096
ROWS_GRP = I_GRP * K                   # 36864 U rows per group (18432 pairs)
SCALE = np.float32(2.0 ** 14)          # keeps fp16 contributions in normal range

F16 = mybir.dt.float16
F32 = mybir.dt.float32
I16 = mybir.dt.int16


# --------------------------------------------------------------------------
# host-side preprocessing: pure index/layout work on the sparse pattern
# --------------------------------------------------------------------------

def _host_prep(inputs):
    qw = np.asarray(inputs["quadrature_weights"], np.float32)
    vals = np.asarray(inputs["psi_vals"], np.float32)
    ik = np.asarray(inputs["psi_idx_k"]).astype(np.int64)
    io = np.asarray(inputs["psi_idx_out"]).astype(np.int64)
    ii = np.asarray(inputs["psi_idx_in"]).astype(np.int64)

    val2 = (vals * qw[ii] * SCALE).astype(np.float32)
    grp = ii >> 12                                   # gather group 0..3
    jloc = (ii & (I_GRP - 1)) * K + ik               # row within group < 36864
    jpair = (jloc >> 1).astype(np.int64)             # 256B pair index < 18432
    parity = (jloc & 1).astype(np.int64)

    core = io >> 11                                  # owning core
    otile = (io >> 7) & (O_TILES - 1)                # o-tile within core
    o_loc = io & (P - 1)

    # sort all entries by (core, o-tile, group, parity); static per-(t,g)
    # slot layout: [GA slots parity0 | GB slots parity1], each 128-padded
    key = (((core * O_TILES + otile) * N_GRP + grp) * 2 + parity)
    order = np.argsort(key, kind="stable")
    key_s = key[order]
    jp_s, ol_s, v_s = jpair[order], o_loc[order], val2[order]
    n_keys = N_CORES * O_TILES * N_GRP * 2
    bounds = np.searchsorted(key_s, np.arange(n_keys + 1))
    counts = (bounds[1:] - bounds[:-1]).reshape(N_CORES, O_TILES, N_GRP, 2)
    GA = int(np.ceil(counts[..., 0].max() / P) * P)
    GB = int(np.ceil(counts[..., 1].max() / P) * P)
    G = GA + GB                                      # slots per (t, g)
    tcols = G // P
    acols = GA // P
    t_slots = N_GRP * G                              # slots per o-tile

    jidx = np.zeros((N_CORES, P, O_TILES * N_GRP * (G // 16)), np.int16)
    ohval = np.zeros((N_CORES, P, O_TILES * N_GRP * tcols * P), np.float16)
    for r in range(N_CORES):
        jp_tg = np.zeros((O_TILES, N_GRP, G), np.int16)
        oh_tg = np.zeros((O_TILES * N_GRP * G, P), np.float16)
        for t in range(O_TILES):
            for g in range(N_GRP):
                base = (t * N_GRP + g) * G
                for par, (off, cap) in enumerate(((0, GA), (GA, GB))):
                    kk = (((r * O_TILES + t) * N_GRP + g) * 2 + par)
                    lo, hi = bounds[kk], bounds[kk + 1]
                    n = hi - lo
                    assert n <= cap, (n, cap)
                    jp_tg[t, g, off:off + n] = jp_s[lo:hi]
                    oh_tg[base + off + np.arange(n), ol_s[lo:hi]] = v_s[lo:hi]
        # idx: wrapped [16, n/16] per (t,g) segment, replicated to 8 Q7 groups
        jw = jp_tg.reshape(O_TILES * N_GRP, G // 16, 16).transpose(2, 0, 1)
        jidx[r] = np.tile(jw.reshape(16, -1), (8, 1))
        # one-hot: slot s -> [s%128, s//128], chunk-major along free dim
        oh = oh_tg.reshape(-1, P, P).transpose(1, 0, 2)   # [128, cols, 128]
        ohval[r] = oh.reshape(P, -1)

    weight = np.asarray(inputs["weight"], np.float32)      # (OC, C, K)
    w16 = weight.transpose(1, 2, 0).reshape(C, K * OC).astype(np.float16)
    w16 = np.ascontiguousarray(np.concatenate([w16, w16], axis=0))  # (64, 288)

    bias = np.asarray(inputs["bias"], np.float32)
    bias_t = np.ascontiguousarray(
        np.broadcast_to(np.tile(bias, B)[None, :], (P, V64))).astype(np.float32)

    x = np.ascontiguousarray(np.asarray(inputs["x"], np.float32))
    common = dict(x=x, wt=w16, biasrow=bias_t)
    percore = [dict(jidx=np.ascontiguousarray(jidx[r]),
                    ohval=np.ascontiguousarray(ohval[r])) for r in range(N_CORES)]
    return percore, common, GA, GB


# --------------------------------------------------------------------------
# device program
# --------------------------------------------------------------------------

def _build(GA, GB):
    G = GA + GB
    tcols = G // P
    acols = GA // P
    gcols16 = G // 16
    nc = bacc.Bacc("TRN2", target_bir_lowering=False, num_swdge_queues=4)

    x_d = nc.dram_tensor("x", [B, C, N_IN], F32, kind="ExternalInput")
    w_d = nc.dram_tensor("wt", [2 * C, K * OC], F16, kind="ExternalInput")
    bias_d = nc.dram_tensor("biasrow", [P, V64], F32, kind="ExternalInput")
    j_d = nc.dram_tensor("jidx", [P, O_TILES * N_GRP * gcols16], I16,
                         kind="ExternalInput")
    oh_d = nc.dram_tensor("ohval", [P, O_TILES * N_GRP * tcols * P], F16,
                          kind="ExternalInput")
    u_d = nc.dram_tensor("U", [N_IN * K, V64], F16, kind="Internal")
    out_d = nc.dram_tensor("out", [O_PER_CORE, V64], F32, kind="ExternalOutput")

    with tile.TileContext(nc) as tc, ExitStack() as ctx:
        cpool = ctx.enter_context(tc.tile_pool(name="const", bufs=1))
        x16 = cpool.tile([2 * C, N_IN], F16)
        nc.gpsimd.dma_start(out=x16[:], in_=x_d[:].rearrange("b c n -> (b c) n"))
        w16 = cpool.tile([2 * C, K * OC], F16)
        nc.sync.dma_start(out=w16[:], in_=w_d[:])
        # staged through a DVE copy so downstream DVE ops read same-engine data
        bias_t0 = cpool.tile([P, V64], F32)
        nc.sync.dma_start(out=bias_t0[:], in_=bias_d[:])
        bias_t = cpool.tile([P, V64], F32)
        nc.vector.tensor_copy(out=bias_t[:], in_=bias_t0[:])

        # ---- U build: U[(i k), (b oc)] = sum_c x16[(b,c), i] w16[(b,c), (k,oc)]
        upool = ctx.enter_context(tc.tile_pool(name="usb", bufs=3))
        ypsum = ctx.enter_context(tc.tile_pool(name="ypsum", bufs=4, space="PSUM"))
        u_ch = u_d[:].rearrange("(n p k) v -> n p (k v)", p=P, k=K)
        for ch in range(N_IN // P):
            u_sb = upool.tile([P, KV], F16)
            u_v = u_sb[:].rearrange("p (k b2 oc) -> p k b2 oc", k=K, b2=B)
            for b in range(B):
                yp = ypsum.tile([P, K * OC], F32)
                nc.tensor.matmul(
                    out=yp[:],
                    lhsT=x16[b * C:(b + 1) * C, ch * P:(ch + 1) * P],
                    rhs=w16[b * C:(b + 1) * C, :],
                    start=True, stop=True)
                nc.vector.tensor_copy(
                    out=u_v[:, :, b, :],
                    in_=yp[:].rearrange("p (k oc) -> p k oc", k=K))
            nc.sync.dma_start(out=u_ch[ch], in_=u_sb[:])

        # every gather reads all of U: collapse the 128 write completions
        tc.strict_bb_all_engine_barrier()

        # ---- sparse gather + segment-sum, one 128-bin o-tile at a time
        ipool = ctx.enter_context(tc.tile_pool(name="idx", bufs=3))
        gpool = ctx.enter_context(tc.tile_pool(name="gath", bufs=2))
        opool = ctx.enter_context(tc.tile_pool(name="ohv", bufs=2))
        opsum = ctx.enter_context(tc.tile_pool(name="opsum", bufs=2, space="PSUM"))
        rpool = ctx.enter_context(tc.tile_pool(name="res", bufs=2))
        u_pair = u_d[:].rearrange("(q two) v -> q (two v)", two=2)  # (73728, 128)
        for t in range(O_TILES):
            jt = ipool.tile([P, N_GRP * gcols16], I16, tag="jt")
            nc.sync.dma_start(
                out=jt[:], in_=j_d[:, t * N_GRP * gcols16:(t + 1) * N_GRP * gcols16])
            oht = opool.tile([P, N_GRP * tcols, P], F16, tag="oht")
            nc.scalar.dma_start(
                out=oht[:], in_=oh_d[:, t * N_GRP * tcols * P:
                                     (t + 1) * N_GRP * tcols * P].rearrange(
                    "p (c o) -> p c o", o=P))

            g = gpool.tile([P, N_GRP * tcols, 2 * V64], F16, tag="g")
            for gr in range(N_GRP):
                # one SWDGE queue per group: each queue is served by its own
                # Q7 core pair, so the 4 desc-gen streams run concurrently
                nc.gpsimd.dma_gather(
                    g[:, gr * tcols:(gr + 1) * tcols, :],
                    u_pair[gr * (ROWS_GRP // 2):(gr + 1) * (ROWS_GRP // 2), :],
                    jt[:, gr * gcols16:(gr + 1) * gcols16],
                    G, G, 2 * V64, elem_step=2 * V64, single_packet=False,
                    queue_num=gr)

            ps = opsum.tile([P, V64], F32)
            n_mm = N_GRP * tcols
            mm = 0
            for gr in range(N_GRP):
                for ci in range(tcols):
                    cc = gr * tcols + ci
                    half = slice(0, V64) if ci < acols else slice(V64, 2 * V64)
                    nc.tensor.matmul(
                        out=ps[:], lhsT=oht[:, cc, :], rhs=g[:, cc, half],
                        start=(mm == 0), stop=(mm == n_mm - 1))
                    mm += 1

            res = rpool.tile([P, V64], F32, tag="res")
            nc.vector.scalar_tensor_tensor(
                out=res[:], in0=ps[:], scalar=float(1.0 / SCALE), in1=bias_t[:],
                op0=mybir.AluOpType.mult, op1=mybir.AluOpType.add)
            res2 = rpool.tile([P, V64], F32, tag="res2")
            nc.vector.tensor_copy(out=res2[:], in_=res[:])
            nc.sync.dma_start(out=out_d[t * P:(t + 1) * P, :], in_=res2[:])

    nc.compile()
    return nc


_last_result = None


def kernel(**inputs) -> np.ndarray:
    global _last_result
    per_core, common, GA, GB = _host_prep(inputs)
    nc = _build(GA, GB)
    in_maps = [{**common, **pc} for pc in per_core]
    r = run_bass_kernel_spmd(nc, in_maps, core_ids=list(range(N_CORES)))
    _last_result = r
    out = np.concatenate([res["out"] for res in r.results], axis=0)  # (16384, 64)
    return np.ascontiguousarray(out.reshape(N_OUT, B, OC).transpose(1, 2, 0))


if __name__ == "__main__":
    rng = np.random.default_rng(0)
    NNZ = 1_500_000
    ins = dict(
        x=rng.standard_normal((B, C, N_IN)).astype(np.float32),
        quadrature_weights=(rng.uniform(0.5, 1.5, N_IN) / N_IN).astype(np.float32),
        psi_vals=rng.uniform(0, 1, NNZ).astype(np.float32),
        weight=(rng.standard_normal((OC, C, K)) / np.sqrt(C)).astype(np.float32),
        bias=np.zeros(OC, np.float32),
        psi_idx_k=rng.integers(0, K, NNZ).astype(np.int32),
        psi_idx_out=rng.integers(0, N_OUT, NNZ).astype(np.int32),
        psi_idx_in=rng.integers(0, N_IN, NNZ).astype(np.int32),
        n_out=N_OUT,
    )
    out = kernel(**ins)
    print("kernel out", out.shape, out.dtype, float(np.abs(out).mean()))
